# revision 4
# baseline (speedup 1.0000x reference)
"""Trainium2 Bass kernel for nn_EqualtimeLayer (spiking-neuron time-to-first-spike).

Math: for each (batch b, postsyn j) the output is the earliest T where
    f(T) = sum_i w[i,j] * relu(T - t[i,j]) >= theta_j,   t[i,j] = s[b,i] + d[i,j]
(first upward threshold crossing of the linear-PSP membrane potential; equivalent
to the reference's sort+cumsum+first-valid-window computation).

Device algorithm (no sort needed):
    f(tau) = sum_i w*max(t,tau) - WTtot          (one fused scalar_tensor_tensor
                                                  probe per column block, with
                                                  free-dim accumulation)
    -> bisection on the monotone predicate f(tau) >= theta, R rounds,
    -> exact finish: T* = lo + (theta + WTtot - S(lo)) / cumW(lo), clamped to
       the final bracket [lo, hi].

Sharding: data-parallel over batch, 4 batches per core on 8 cores. Weights and
delays are transposed once on the host (j-major layout) so each probe is a
per-partition-scalar op with j on partitions and i on the free axis.
"""

import numpy as np

import concourse.bacc as bacc
import concourse.mybir as mybir
import concourse.tile as tile
from concourse.bass_utils import run_bass_kernel_spmd

F32 = mybir.dt.float32
U8 = mybir.dt.uint8
ALU = mybir.AluOpType

B, PRE, POST = 32, 1024, 1024
N_CORES = 8
B_LOC = B // N_CORES          # 4 batches per core
JB = POST // 128              # 8 j-blocks of 128 partitions
NCOL = B_LOC * JB             # 32 state columns, col = b*JB + jb
R_BISECT = 21                 # bracket width 2/2^21 ~ 1e-6


def _build(R=R_BISECT, finish=True, infguard=True):
    nc = bacc.Bacc("TRN2", target_bir_lowering=False, debug=False)

    dT = nc.dram_tensor("dT", [POST, PRE], F32, kind="ExternalInput")      # d transposed [j, i]
    wT = nc.dram_tensor("wT", [POST, PRE], F32, kind="ExternalInput")      # w transposed [j, i]
    s_loc = nc.dram_tensor("s_loc", [B_LOC, PRE], F32, kind="ExternalInput")
    th = nc.dram_tensor("th", [POST], F32, kind="ExternalInput")
    out_loc = nc.dram_tensor("out_loc", [B_LOC, POST], F32, kind="ExternalOutput")

    with tile.TileContext(nc) as tc:
        with (
            tc.tile_pool(name="big", bufs=1) as big,
            tc.tile_pool(name="mat", bufs=1) as mat,
            tc.tile_pool(name="midp", bufs=2) as midp,
            tc.tile_pool(name="small", bufs=1) as small,
        ):
            # ---- load d^T into a shared slot, build t^T[b] = d^T + s[b] ----
            dwT = mat.tile([128, JB, PRE], F32, tag="dw", name="dT_t")   # holds d^T now, w^T later
            for jb in range(JB):
                nc.sync.dma_start(out=dwT[:, jb, :], in_=dT[jb * 128:(jb + 1) * 128, :])

            tT = []
            for b in range(B_LOC):
                tT.append(big.tile([128, JB, PRE], F32, tag=f"tT{b}", name=f"tT{b}"))

            for b in range(B_LOC):
                srep = midp.tile([128, PRE], F32, tag="srep", name=f"srep{b}")
                nc.sync.dma_start(out=srep[:], in_=s_loc[b:b + 1, :].partition_broadcast(128))
                for jb in range(JB):
                    nc.vector.tensor_tensor(
                        out=tT[b][:, jb, :], in0=dwT[:, jb, :], in1=srep[:], op=ALU.add)

            # d^T no longer needed -> reuse the slot for w^T (same tag, bufs=1)
            wTt = mat.tile([128, JB, PRE], F32, tag="dw", name="wT_t")
            for jb in range(JB):
                nc.sync.dma_start(out=wTt[:, jb, :], in_=wT[jb * 128:(jb + 1) * 128, :])

            # thresholds -> [128, JB]
            th_t = small.tile([128, JB], F32, tag="th", name="th_t")
            nc.sync.dma_start(out=th_t[:], in_=th.rearrange("(jb p) -> p jb", p=128))

            # ---- per-column state [128, NCOL], col = b*JB + jb ----
            def st(tag, dt=F32):
                return small.tile([128, NCOL], dt, tag=tag, name=tag)

            lo, hi, mid, S, thW = st("lo"), st("hi"), st("mid"), st("S"), st("thW")
            pred_ge, pred_lt = st("pge", U8), st("plt", U8)
            scr0 = st("scr0")

            def probe(scalar_tile, op0, acc_tile):
                """acc[:, col] = sum_i (t^T[b,jb] op0 scalar[col]) * w^T[jb]"""
                for b in range(B_LOC):
                    for jb in range(JB):
                        col = b * JB + jb
                        scratch = midp.tile([128, PRE], F32, tag="scratch", name="scratch")
                        nc.vector.scalar_tensor_tensor(
                            out=scratch[:],
                            in0=tT[b][:, jb, :],
                            scalar=scalar_tile[:, col:col + 1],
                            in1=wTt[:, jb, :],
                            op0=op0, op1=ALU.mult,
                            accum_out=acc_tile[:, col:col + 1])

            # WTtot = sum w*t  (= probe with tau=0 since all t > 0); thW = th + WTtot
            nc.vector.memset(mid[:], 0.0)
            probe(mid, ALU.max, S)
            for b in range(B_LOC):
                nc.vector.tensor_tensor(
                    out=thW[:, b * JB:(b + 1) * JB], in0=S[:, b * JB:(b + 1) * JB],
                    in1=th_t[:], op=ALU.add)

            # ---- bisection ----
            nc.vector.memset(lo[:], 0.0)
            nc.vector.memset(hi[:], 2.0)
            for _ in range(R):
                nc.vector.tensor_tensor(out=scr0[:], in0=lo[:], in1=hi[:], op=ALU.add)
                nc.vector.tensor_scalar_mul(mid[:], scr0[:], 0.5)
                probe(mid, ALU.max, S)
                nc.vector.tensor_tensor(out=pred_ge[:], in0=S[:], in1=thW[:], op=ALU.is_ge)
                nc.vector.tensor_tensor(out=pred_lt[:], in0=S[:], in1=thW[:], op=ALU.is_lt)
                nc.vector.copy_predicated(out=hi[:], mask=pred_ge[:], data=mid[:])
                nc.vector.copy_predicated(out=lo[:], mask=pred_lt[:], data=mid[:])

            # ---- exact finish: c = lo + (thW - S(lo)) / cumW(lo), clamp to [lo, hi] ----
            cumw, num, rec, cand = st("cumw"), st("num"), st("rec"), st("cand")
            if finish:
                probe(lo, ALU.max, S)
                probe(lo, ALU.is_le, cumw)
                nc.vector.tensor_tensor(out=num[:], in0=thW[:], in1=S[:], op=ALU.subtract)
                nc.vector.reciprocal(out=rec[:], in_=cumw[:])
                nc.vector.tensor_tensor(out=scr0[:], in0=num[:], in1=rec[:], op=ALU.mult)
                nc.vector.tensor_tensor(out=cand[:], in0=scr0[:], in1=lo[:], op=ALU.add)
                nc.vector.tensor_tensor(out=scr0[:], in0=cand[:], in1=lo[:], op=ALU.max)
                nc.vector.tensor_tensor(out=cand[:], in0=scr0[:], in1=hi[:], op=ALU.min)
            else:
                nc.vector.tensor_tensor(out=cand[:], in0=lo[:], in1=hi[:], op=ALU.add)
            if infguard:
                # never-crossed columns (hi still == 2.0) -> +inf like the reference
                infs = st("infs")
                nc.vector.memset(infs[:], float("inf"))
                nc.vector.tensor_scalar(out=pred_ge[:], in0=hi[:], scalar1=2.0, scalar2=None,
                                        op0=ALU.is_ge)
                nc.vector.copy_predicated(out=cand[:], mask=pred_ge[:], data=infs[:])

            for b in range(B_LOC):
                nc.sync.dma_start(
                    out=out_loc[b].rearrange("(jb p) -> p jb", p=128),
                    in_=cand[:, b * JB:(b + 1) * JB])

    nc.compile()
    return nc


_NC_CACHE = None


def kernel(input_spikes, input_weights, input_delays, thresholds):
    global _NC_CACHE
    if _NC_CACHE is None:
        _NC_CACHE = _build()
    nc = _NC_CACHE

    s = np.ascontiguousarray(input_spikes, dtype=np.float32)
    wT = np.ascontiguousarray(np.asarray(input_weights, dtype=np.float32).T)
    dT = np.ascontiguousarray(np.asarray(input_delays, dtype=np.float32).T)
    th = np.ascontiguousarray(thresholds, dtype=np.float32)

    in_maps = [
        dict(dT=dT, wT=wT, s_loc=np.ascontiguousarray(s[k * B_LOC:(k + 1) * B_LOC]), th=th)
        for k in range(N_CORES)
    ]
    res = run_bass_kernel_spmd(nc, in_maps, core_ids=list(range(N_CORES)))
    out = np.concatenate([r["out_loc"] for r in res.results], axis=0)
    return out.astype(np.float32)


if __name__ == "__main__":
    rng = np.random.default_rng(0)
    s = rng.uniform(0, 1, (B, PRE)).astype(np.float32)
    w = (rng.normal(0, 1, (PRE, POST)) * 0.1 + 0.05).astype(np.float32)
    d = rng.uniform(0, 1, (PRE, POST)).astype(np.float32)
    th = np.ones(POST, np.float32)
    out = kernel(s, w, d, th)
    print("out", out.shape, out.dtype, np.percentile(out[np.isfinite(out)], [0, 50, 100]))


# revision 6
# speedup vs baseline: 1.8459x; 1.8459x over previous
"""Trainium2 Bass kernel for nn_EqualtimeLayer (spiking-neuron time-to-first-spike).

Math: for each (batch b, postsyn j) the output is the earliest T where
    f(T) = sum_i w[i,j] * relu(T - t[i,j]) >= theta_j,   t[i,j] = s[b,i] + d[i,j]
(first upward threshold crossing of the linear-PSP membrane potential; equivalent
to the reference's sort+cumsum+first-valid-window computation).

Device algorithm (no sort needed):
    f(tau) = sum_i w*max(t,tau) - WTtot          (one fused scalar_tensor_tensor
                                                  probe per column block, with
                                                  free-dim accumulation)
    -> bisection on the monotone predicate f(tau) >= theta, R rounds,
    -> exact finish: T* = lo + (theta + WTtot - S(lo)) / cumW(lo), clamped to
       the final bracket [lo, hi].

Sharding: data-parallel over batch, 4 batches per core on 8 cores. Weights and
delays are transposed once on the host (j-major layout) so each probe is a
per-partition-scalar op with j on partitions and i on the free axis.
"""

import numpy as np

import concourse.bacc as bacc
import concourse.mybir as mybir
import concourse.tile as tile
from concourse.bass_utils import run_bass_kernel_spmd

F32 = mybir.dt.float32
U8 = mybir.dt.uint8
ALU = mybir.AluOpType

B, PRE, POST = 32, 1024, 1024
N_CORES = 8
B_LOC = B // N_CORES          # 4 batches per core
JB = POST // 128              # 8 j-blocks of 128 partitions
NCOL = B_LOC * JB             # 32 state columns, col = b*JB + jb
R_BISECT = 6                  # coarse bracket, then Newton steps
K_NEWTON = 3


def _build(R=R_BISECT, infguard=True):
    nc = bacc.Bacc("TRN2", target_bir_lowering=False, debug=False)

    dT = nc.dram_tensor("dT", [POST, PRE], F32, kind="ExternalInput")      # d transposed [j, i]
    wT = nc.dram_tensor("wT", [POST, PRE], F32, kind="ExternalInput")      # w transposed [j, i]
    s_loc = nc.dram_tensor("s_loc", [B_LOC, PRE], F32, kind="ExternalInput")
    th = nc.dram_tensor("th", [POST], F32, kind="ExternalInput")
    out_loc = nc.dram_tensor("out_loc", [B_LOC, POST], F32, kind="ExternalOutput")

    with tile.TileContext(nc) as tc:
        with (
            tc.tile_pool(name="big", bufs=1) as big,
            tc.tile_pool(name="mat", bufs=1) as mat,
            tc.tile_pool(name="midp", bufs=2) as midp,
            tc.tile_pool(name="small", bufs=1) as small,
        ):
            # ---- load d^T into a shared slot, build t^T[b] = d^T + s[b] ----
            dwT = mat.tile([128, JB, PRE], F32, tag="dw", name="dT_t")   # holds d^T now, w^T later
            for jb in range(JB):
                nc.sync.dma_start(out=dwT[:, jb, :], in_=dT[jb * 128:(jb + 1) * 128, :])

            tT = []
            for b in range(B_LOC):
                tT.append(big.tile([128, JB, PRE], F32, tag=f"tT{b}", name=f"tT{b}"))

            for b in range(B_LOC):
                srep = midp.tile([128, PRE], F32, tag="srep", name=f"srep{b}")
                nc.sync.dma_start(out=srep[:], in_=s_loc[b:b + 1, :].partition_broadcast(128))
                for jb in range(JB):
                    nc.vector.tensor_tensor(
                        out=tT[b][:, jb, :], in0=dwT[:, jb, :], in1=srep[:], op=ALU.add)

            # d^T no longer needed -> reuse the slot for w^T (same tag, bufs=1)
            wTt = mat.tile([128, JB, PRE], F32, tag="dw", name="wT_t")
            for jb in range(JB):
                nc.sync.dma_start(out=wTt[:, jb, :], in_=wT[jb * 128:(jb + 1) * 128, :])

            # thresholds -> [128, JB]
            th_t = small.tile([128, JB], F32, tag="th", name="th_t")
            nc.sync.dma_start(out=th_t[:], in_=th.rearrange("(jb p) -> p jb", p=128))

            # ---- per-column state [128, NCOL], col = b*JB + jb ----
            def st(tag, dt=F32):
                return small.tile([128, NCOL], dt, tag=tag, name=tag)

            lo, hi, mid, S, thW = st("lo"), st("hi"), st("mid"), st("S"), st("thW")
            pred_ge, pred_lt = st("pge", U8), st("plt", U8)
            scr0 = st("scr0")

            def probe(scalar_tile, op0, acc_tile):
                """acc[:, col] = sum_i (t^T[b,jb] op0 scalar[col]) * w^T[jb]"""
                for b in range(B_LOC):
                    for jb in range(JB):
                        col = b * JB + jb
                        scratch = midp.tile([128, PRE], F32, tag="scratch", name="scratch")
                        nc.vector.scalar_tensor_tensor(
                            out=scratch[:],
                            in0=tT[b][:, jb, :],
                            scalar=scalar_tile[:, col:col + 1],
                            in1=wTt[:, jb, :],
                            op0=op0, op1=ALU.mult,
                            accum_out=acc_tile[:, col:col + 1])

            # WTtot = sum w*t  (= probe with tau=0 since all t > 0); thW = th + WTtot
            nc.vector.memset(mid[:], 0.0)
            probe(mid, ALU.max, S)
            for b in range(B_LOC):
                nc.vector.tensor_tensor(
                    out=thW[:, b * JB:(b + 1) * JB], in0=S[:, b * JB:(b + 1) * JB],
                    in1=th_t[:], op=ALU.add)

            # ---- bisection (coarse bracket) ----
            nc.vector.memset(lo[:], 0.0)
            nc.vector.memset(hi[:], 2.0)
            for _ in range(R):
                nc.vector.tensor_tensor(out=scr0[:], in0=lo[:], in1=hi[:], op=ALU.add)
                nc.vector.tensor_scalar_mul(mid[:], scr0[:], 0.5)
                probe(mid, ALU.max, S)
                nc.vector.tensor_tensor(out=pred_ge[:], in0=S[:], in1=thW[:], op=ALU.is_ge)
                nc.vector.tensor_tensor(out=pred_lt[:], in0=S[:], in1=thW[:], op=ALU.is_lt)
                nc.vector.copy_predicated(out=hi[:], mask=pred_ge[:], data=mid[:])
                nc.vector.copy_predicated(out=lo[:], mask=pred_lt[:], data=mid[:])

            # ---- Newton-finish: tau <- clamp(tau + (thW - S(tau))/cumW(tau), [lo,hi]) ----
            # step 1 reuses the last bisection round's S(mid); tau starts at mid.
            cumw, num, rec, cand = st("cumw"), st("num"), st("rec"), st("cand")
            tau, scr1 = st("tau"), st("scr1")
            nc.vector.tensor_copy(tau[:], mid[:])
            for k in range(K_NEWTON):
                if k > 0:
                    probe(tau, ALU.max, S)
                    nc.vector.tensor_tensor(out=pred_ge[:], in0=S[:], in1=thW[:], op=ALU.is_ge)
                    nc.vector.tensor_tensor(out=pred_lt[:], in0=S[:], in1=thW[:], op=ALU.is_lt)
                    nc.vector.tensor_tensor(out=scr0[:], in0=tau[:], in1=hi[:], op=ALU.min)
                    nc.vector.tensor_tensor(out=scr1[:], in0=tau[:], in1=lo[:], op=ALU.max)
                    nc.vector.copy_predicated(out=hi[:], mask=pred_ge[:], data=scr0[:])
                    nc.vector.copy_predicated(out=lo[:], mask=pred_lt[:], data=scr1[:])
                probe(tau, ALU.is_le, cumw)
                nc.vector.tensor_tensor(out=num[:], in0=thW[:], in1=S[:], op=ALU.subtract)
                nc.vector.reciprocal(out=rec[:], in_=cumw[:])
                nc.vector.tensor_tensor(out=scr0[:], in0=num[:], in1=rec[:], op=ALU.mult)
                nc.vector.tensor_tensor(out=scr1[:], in0=scr0[:], in1=tau[:], op=ALU.add)
                nc.vector.tensor_tensor(out=scr0[:], in0=scr1[:], in1=lo[:], op=ALU.max)
                nc.vector.tensor_tensor(out=tau[:], in0=scr0[:], in1=hi[:], op=ALU.min)
            nc.vector.tensor_copy(cand[:], tau[:])
            if infguard:
                # never-crossed columns (hi still == 2.0) -> +inf like the reference
                infs = st("infs")
                nc.vector.memset(infs[:], float("inf"))
                nc.vector.tensor_scalar(out=pred_ge[:], in0=hi[:], scalar1=2.0, scalar2=None,
                                        op0=ALU.is_ge)
                nc.vector.copy_predicated(out=cand[:], mask=pred_ge[:], data=infs[:])

            for b in range(B_LOC):
                nc.sync.dma_start(
                    out=out_loc[b].rearrange("(jb p) -> p jb", p=128),
                    in_=cand[:, b * JB:(b + 1) * JB])

    nc.compile()
    return nc


_NC_CACHE = None


def kernel(input_spikes, input_weights, input_delays, thresholds):
    global _NC_CACHE
    if _NC_CACHE is None:
        _NC_CACHE = _build()
    nc = _NC_CACHE

    s = np.ascontiguousarray(input_spikes, dtype=np.float32)
    wT = np.ascontiguousarray(np.asarray(input_weights, dtype=np.float32).T)
    dT = np.ascontiguousarray(np.asarray(input_delays, dtype=np.float32).T)
    th = np.ascontiguousarray(thresholds, dtype=np.float32)

    in_maps = [
        dict(dT=dT, wT=wT, s_loc=np.ascontiguousarray(s[k * B_LOC:(k + 1) * B_LOC]), th=th)
        for k in range(N_CORES)
    ]
    res = run_bass_kernel_spmd(nc, in_maps, core_ids=list(range(N_CORES)))
    out = np.concatenate([r["out_loc"] for r in res.results], axis=0)
    return out.astype(np.float32)


if __name__ == "__main__":
    rng = np.random.default_rng(0)
    s = rng.uniform(0, 1, (B, PRE)).astype(np.float32)
    w = (rng.normal(0, 1, (PRE, POST)) * 0.1 + 0.05).astype(np.float32)
    d = rng.uniform(0, 1, (PRE, POST)).astype(np.float32)
    th = np.ones(POST, np.float32)
    out = kernel(s, w, d, th)
    print("out", out.shape, out.dtype, np.percentile(out[np.isfinite(out)], [0, 50, 100]))


# revision 7
# speedup vs baseline: 2.0066x; 1.0871x over previous
"""Trainium2 Bass kernel for nn_EqualtimeLayer (spiking-neuron time-to-first-spike).

Math: for each (batch b, postsyn j) the output is the earliest T where
    f(T) = sum_i w[i,j] * relu(T - t[i,j]) >= theta_j,   t[i,j] = s[b,i] + d[i,j]
(first upward threshold crossing of the linear-PSP membrane potential; equivalent
to the reference's sort+cumsum+first-valid-window computation).

Device algorithm (no sort needed):
    f(tau) = sum_i w*max(t,tau) - WTtot          (one fused scalar_tensor_tensor
                                                  probe per column block, with
                                                  free-dim accumulation)
    -> bisection on the monotone predicate f(tau) >= theta, R rounds,
    -> exact finish: T* = lo + (theta + WTtot - S(lo)) / cumW(lo), clamped to
       the final bracket [lo, hi].

Sharding: data-parallel over batch, 4 batches per core on 8 cores. Weights and
delays are transposed once on the host (j-major layout) so each probe is a
per-partition-scalar op with j on partitions and i on the free axis.
"""

import numpy as np

import concourse.bacc as bacc
import concourse.mybir as mybir
import concourse.tile as tile
from concourse.bass_utils import run_bass_kernel_spmd

F32 = mybir.dt.float32
U8 = mybir.dt.uint8
ALU = mybir.AluOpType

B, PRE, POST = 32, 1024, 1024
N_CORES = 8
B_LOC = B // N_CORES          # 4 batches per core
JB = POST // 128              # 8 j-blocks of 128 partitions
NCOL = B_LOC * JB             # 32 state columns, col = b*JB + jb
R_BISECT = 5                  # coarse bracket, then Newton steps
K_NEWTON = 3


def _build(R=R_BISECT, infguard=True):
    nc = bacc.Bacc("TRN2", target_bir_lowering=False, debug=False)

    dT = nc.dram_tensor("dT", [POST, PRE], F32, kind="ExternalInput")      # d transposed [j, i]
    wT = nc.dram_tensor("wT", [POST, PRE], F32, kind="ExternalInput")      # w transposed [j, i]
    s_loc = nc.dram_tensor("s_loc", [B_LOC, PRE], F32, kind="ExternalInput")
    th = nc.dram_tensor("th", [POST], F32, kind="ExternalInput")
    out_loc = nc.dram_tensor("out_loc", [B_LOC, POST], F32, kind="ExternalOutput")

    with tile.TileContext(nc) as tc:
        with (
            tc.tile_pool(name="big", bufs=1) as big,
            tc.tile_pool(name="mat", bufs=1) as mat,
            tc.tile_pool(name="midp", bufs=2) as midp,
            tc.tile_pool(name="small", bufs=1) as small,
        ):
            # ---- load d^T (per-jb slots), build t^T[b] = d^T + s[b] ----
            # w^T reuses each jb slot as soon as that jb's t-builds finish, so the
            # w^T DMA pipelines with the t-build instead of waiting for all of it.
            dw = [mat.tile([128, PRE], F32, tag=f"dw{jb}", name=f"dT{jb}") for jb in range(JB)]
            for jb in range(JB):
                nc.sync.dma_start(out=dw[jb][:], in_=dT[jb * 128:(jb + 1) * 128, :])

            tT = []
            for b in range(B_LOC):
                tT.append(big.tile([128, JB, PRE], F32, tag=f"tT{b}", name=f"tT{b}"))

            sreps = []
            for b in range(B_LOC):
                srep = midp.tile([128, PRE], F32, tag=f"srep{b % 2}", name=f"srep{b}")
                nc.sync.dma_start(out=srep[:], in_=s_loc[b:b + 1, :].partition_broadcast(128))
                sreps.append(srep)
            for jb in range(JB):
                for b in range(B_LOC):
                    nc.vector.tensor_tensor(
                        out=tT[b][:, jb, :], in0=dw[jb][:], in1=sreps[b][:], op=ALU.add)

            wTt_tiles = [mat.tile([128, PRE], F32, tag=f"dw{jb}", name=f"wT{jb}") for jb in range(JB)]
            for jb in range(JB):
                nc.sync.dma_start(out=wTt_tiles[jb][:], in_=wT[jb * 128:(jb + 1) * 128, :])

            # thresholds -> [128, JB]
            th_t = small.tile([128, JB], F32, tag="th", name="th_t")
            nc.sync.dma_start(out=th_t[:], in_=th.rearrange("(jb p) -> p jb", p=128))

            # ---- per-column state [128, NCOL], col = b*JB + jb ----
            def st(tag, dt=F32):
                return small.tile([128, NCOL], dt, tag=tag, name=tag)

            lo, hi, mid, S, thW = st("lo"), st("hi"), st("mid"), st("S"), st("thW")
            pred_ge, pred_lt = st("pge", U8), st("plt", U8)
            scr0 = st("scr0")

            def probe(scalar_tile, op0, acc_tile):
                """acc[:, col] = sum_i (t^T[b,jb] op0 scalar[col]) * w^T[jb]"""
                for b in range(B_LOC):
                    for jb in range(JB):
                        col = b * JB + jb
                        scratch = midp.tile([128, PRE], F32, tag="scratch", name="scratch")
                        nc.vector.scalar_tensor_tensor(
                            out=scratch[:],
                            in0=tT[b][:, jb, :],
                            scalar=scalar_tile[:, col:col + 1],
                            in1=wTt_tiles[jb][:],
                            op0=op0, op1=ALU.mult,
                            accum_out=acc_tile[:, col:col + 1])

            # WTtot = sum w*t  (= probe with tau=0 since all t > 0); thW = th + WTtot
            nc.vector.memset(mid[:], 0.0)
            probe(mid, ALU.max, S)
            for b in range(B_LOC):
                nc.vector.tensor_tensor(
                    out=thW[:, b * JB:(b + 1) * JB], in0=S[:, b * JB:(b + 1) * JB],
                    in1=th_t[:], op=ALU.add)

            # ---- bisection (coarse bracket) ----
            nc.vector.memset(lo[:], 0.0)
            nc.vector.memset(hi[:], 2.0)
            for _ in range(R):
                nc.vector.tensor_tensor(out=scr0[:], in0=lo[:], in1=hi[:], op=ALU.add)
                nc.vector.tensor_scalar_mul(mid[:], scr0[:], 0.5)
                probe(mid, ALU.max, S)
                nc.vector.tensor_tensor(out=pred_ge[:], in0=S[:], in1=thW[:], op=ALU.is_ge)
                nc.vector.tensor_tensor(out=pred_lt[:], in0=S[:], in1=thW[:], op=ALU.is_lt)
                nc.vector.copy_predicated(out=hi[:], mask=pred_ge[:], data=mid[:])
                nc.vector.copy_predicated(out=lo[:], mask=pred_lt[:], data=mid[:])

            # ---- Newton-finish: tau <- clamp(tau + (thW - S(tau))/cumW(tau), [lo,hi]) ----
            # step 1 reuses the last bisection round's S(mid); tau starts at mid.
            cumw, num, rec, cand = st("cumw"), st("num"), st("rec"), st("cand")
            tau, scr1 = st("tau"), st("scr1")
            nc.vector.tensor_copy(tau[:], mid[:])
            for k in range(K_NEWTON):
                if k > 0:
                    probe(tau, ALU.max, S)
                    nc.vector.tensor_tensor(out=pred_ge[:], in0=S[:], in1=thW[:], op=ALU.is_ge)
                    nc.vector.tensor_tensor(out=pred_lt[:], in0=S[:], in1=thW[:], op=ALU.is_lt)
                    nc.vector.tensor_tensor(out=scr0[:], in0=tau[:], in1=hi[:], op=ALU.min)
                    nc.vector.tensor_tensor(out=scr1[:], in0=tau[:], in1=lo[:], op=ALU.max)
                    nc.vector.copy_predicated(out=hi[:], mask=pred_ge[:], data=scr0[:])
                    nc.vector.copy_predicated(out=lo[:], mask=pred_lt[:], data=scr1[:])
                probe(tau, ALU.is_le, cumw)
                nc.vector.tensor_tensor(out=num[:], in0=thW[:], in1=S[:], op=ALU.subtract)
                nc.vector.reciprocal(out=rec[:], in_=cumw[:])
                nc.vector.tensor_tensor(out=scr0[:], in0=num[:], in1=rec[:], op=ALU.mult)
                nc.vector.tensor_tensor(out=scr1[:], in0=scr0[:], in1=tau[:], op=ALU.add)
                nc.vector.tensor_tensor(out=scr0[:], in0=scr1[:], in1=lo[:], op=ALU.max)
                nc.vector.tensor_tensor(out=tau[:], in0=scr0[:], in1=hi[:], op=ALU.min)
            nc.vector.tensor_copy(cand[:], tau[:])
            if infguard:
                # never-crossed columns (hi still == 2.0) -> +inf like the reference
                infs = st("infs")
                nc.vector.memset(infs[:], float("inf"))
                nc.vector.tensor_scalar(out=pred_ge[:], in0=hi[:], scalar1=2.0, scalar2=None,
                                        op0=ALU.is_ge)
                nc.vector.copy_predicated(out=cand[:], mask=pred_ge[:], data=infs[:])

            for b in range(B_LOC):
                nc.sync.dma_start(
                    out=out_loc[b].rearrange("(jb p) -> p jb", p=128),
                    in_=cand[:, b * JB:(b + 1) * JB])

    nc.compile()
    return nc


_NC_CACHE = None


def kernel(input_spikes, input_weights, input_delays, thresholds):
    global _NC_CACHE
    if _NC_CACHE is None:
        _NC_CACHE = _build()
    nc = _NC_CACHE

    s = np.ascontiguousarray(input_spikes, dtype=np.float32)
    wT = np.ascontiguousarray(np.asarray(input_weights, dtype=np.float32).T)
    dT = np.ascontiguousarray(np.asarray(input_delays, dtype=np.float32).T)
    th = np.ascontiguousarray(thresholds, dtype=np.float32)

    in_maps = [
        dict(dT=dT, wT=wT, s_loc=np.ascontiguousarray(s[k * B_LOC:(k + 1) * B_LOC]), th=th)
        for k in range(N_CORES)
    ]
    res = run_bass_kernel_spmd(nc, in_maps, core_ids=list(range(N_CORES)))
    out = np.concatenate([r["out_loc"] for r in res.results], axis=0)
    return out.astype(np.float32)


if __name__ == "__main__":
    rng = np.random.default_rng(0)
    s = rng.uniform(0, 1, (B, PRE)).astype(np.float32)
    w = (rng.normal(0, 1, (PRE, POST)) * 0.1 + 0.05).astype(np.float32)
    d = rng.uniform(0, 1, (PRE, POST)).astype(np.float32)
    th = np.ones(POST, np.float32)
    out = kernel(s, w, d, th)
    print("out", out.shape, out.dtype, np.percentile(out[np.isfinite(out)], [0, 50, 100]))


# revision 8
# speedup vs baseline: 2.0865x; 1.0398x over previous
"""Trainium2 Bass kernel for nn_EqualtimeLayer (spiking-neuron time-to-first-spike).

Math: for each (batch b, postsyn j) the output is the earliest T where
    f(T) = sum_i w[i,j] * relu(T - t[i,j]) >= theta_j,   t[i,j] = s[b,i] + d[i,j]
(first upward threshold crossing of the linear-PSP membrane potential; equivalent
to the reference's sort+cumsum+first-valid-window computation).

Device algorithm (no sort needed):
    f(tau) = sum_i w*max(t,tau) - WTtot          (one fused scalar_tensor_tensor
                                                  probe per column block, with
                                                  free-dim accumulation)
    -> bisection on the monotone predicate f(tau) >= theta, R rounds,
    -> exact finish: T* = lo + (theta + WTtot - S(lo)) / cumW(lo), clamped to
       the final bracket [lo, hi].

Sharding: data-parallel over batch, 4 batches per core on 8 cores. Weights and
delays are transposed once on the host (j-major layout) so each probe is a
per-partition-scalar op with j on partitions and i on the free axis.
"""

import numpy as np

import concourse.bacc as bacc
import concourse.mybir as mybir
import concourse.tile as tile
from concourse.bass_utils import run_bass_kernel_spmd

F32 = mybir.dt.float32
U8 = mybir.dt.uint8
ALU = mybir.AluOpType

B, PRE, POST = 32, 1024, 1024
N_CORES = 8
B_LOC = B // N_CORES          # 4 batches per core
JB = POST // 128              # 8 j-blocks of 128 partitions
NCOL = B_LOC * JB             # 32 state columns, col = b*JB + jb
R_BISECT = 5                  # coarse bracket, then Newton steps
K_NEWTON = 3


def _build(R=R_BISECT, infguard=True):
    nc = bacc.Bacc("TRN2", target_bir_lowering=False, debug=False)

    dT = nc.dram_tensor("dT", [POST, PRE], F32, kind="ExternalInput")      # d transposed [j, i]
    wT = nc.dram_tensor("wT", [POST, PRE], F32, kind="ExternalInput")      # w transposed [j, i]
    s_loc = nc.dram_tensor("s_loc", [B_LOC, PRE], F32, kind="ExternalInput")
    thw_in = nc.dram_tensor("thw_in", [B_LOC, POST], F32, kind="ExternalInput")
    out_loc = nc.dram_tensor("out_loc", [B_LOC, POST], F32, kind="ExternalOutput")

    with tile.TileContext(nc) as tc:
        with (
            tc.tile_pool(name="big", bufs=1) as big,
            tc.tile_pool(name="mat", bufs=1) as mat,
            tc.tile_pool(name="midp", bufs=2) as midp,
            tc.tile_pool(name="small", bufs=1) as small,
        ):
            # ---- load d^T (per-jb slots), build t^T[b] = d^T + s[b] ----
            # w^T reuses each jb slot as soon as that jb's t-builds finish, so the
            # w^T DMA pipelines with the t-build instead of waiting for all of it.
            dw = [mat.tile([128, PRE], F32, tag=f"dw{jb}", name=f"dT{jb}") for jb in range(JB)]
            for jb in range(JB):
                nc.sync.dma_start(out=dw[jb][:], in_=dT[jb * 128:(jb + 1) * 128, :])

            tT = []
            for b in range(B_LOC):
                tT.append(big.tile([128, JB, PRE], F32, tag=f"tT{b}", name=f"tT{b}"))

            sreps = []
            for b in range(B_LOC):
                srep = midp.tile([128, PRE], F32, tag=f"srep{b % 2}", name=f"srep{b}")
                nc.sync.dma_start(out=srep[:], in_=s_loc[b:b + 1, :].partition_broadcast(128))
                sreps.append(srep)
            for jb in range(JB):
                for b in range(B_LOC):
                    eng = nc.gpsimd if b >= 2 else nc.vector
                    eng.tensor_tensor(
                        out=tT[b][:, jb, :], in0=dw[jb][:], in1=sreps[b][:], op=ALU.add)

            wTt_tiles = [mat.tile([128, PRE], F32, tag=f"dw{jb}", name=f"wT{jb}") for jb in range(JB)]
            for jb in range(JB):
                nc.sync.dma_start(out=wTt_tiles[jb][:], in_=wT[jb * 128:(jb + 1) * 128, :])

            # ---- per-column state [128, NCOL], col = b*JB + jb ----
            def st(tag, dt=F32):
                return small.tile([128, NCOL], dt, tag=tag, name=tag)

            lo, hi, mid, S, thW = st("lo"), st("hi"), st("mid"), st("S"), st("thW")
            pred_ge, pred_lt = st("pge", U8), st("plt", U8)
            scr0 = st("scr0")

            def probe(scalar_tile, op0, acc_tile):
                """acc[:, col] = sum_i (t^T[b,jb] op0 scalar[col]) * w^T[jb]"""
                for b in range(B_LOC):
                    for jb in range(JB):
                        col = b * JB + jb
                        scratch = midp.tile([128, PRE], F32, tag="scratch", name="scratch")
                        nc.vector.scalar_tensor_tensor(
                            out=scratch[:],
                            in0=tT[b][:, jb, :],
                            scalar=scalar_tile[:, col:col + 1],
                            in1=wTt_tiles[jb][:],
                            op0=op0, op1=ALU.mult,
                            accum_out=acc_tile[:, col:col + 1])

            # thW = th + sum_i w*t, computed on the host (GEMM) and loaded directly
            # into the state layout [128, col] (col = b*JB + jb).
            for b in range(B_LOC):
                nc.sync.dma_start(
                    out=thW[:, b * JB:(b + 1) * JB],
                    in_=thw_in[b].rearrange("(jb p) -> p jb", p=128))

            # ---- bisection (coarse bracket) ----
            nc.vector.memset(lo[:], 0.0)
            nc.vector.memset(hi[:], 2.0)
            for _ in range(R):
                nc.vector.tensor_tensor(out=scr0[:], in0=lo[:], in1=hi[:], op=ALU.add)
                nc.vector.tensor_scalar_mul(mid[:], scr0[:], 0.5)
                probe(mid, ALU.max, S)
                nc.vector.tensor_tensor(out=pred_ge[:], in0=S[:], in1=thW[:], op=ALU.is_ge)
                nc.vector.tensor_tensor(out=pred_lt[:], in0=S[:], in1=thW[:], op=ALU.is_lt)
                nc.vector.copy_predicated(out=hi[:], mask=pred_ge[:], data=mid[:])
                nc.vector.copy_predicated(out=lo[:], mask=pred_lt[:], data=mid[:])

            # ---- Newton-finish: tau <- clamp(tau + (thW - S(tau))/cumW(tau), [lo,hi]) ----
            # step 1 reuses the last bisection round's S(mid); tau starts at mid.
            cumw, num, rec, cand = st("cumw"), st("num"), st("rec"), st("cand")
            tau, scr1 = st("tau"), st("scr1")
            nc.vector.tensor_copy(tau[:], mid[:])
            for k in range(K_NEWTON):
                if k > 0:
                    probe(tau, ALU.max, S)
                    nc.vector.tensor_tensor(out=pred_ge[:], in0=S[:], in1=thW[:], op=ALU.is_ge)
                    nc.vector.tensor_tensor(out=pred_lt[:], in0=S[:], in1=thW[:], op=ALU.is_lt)
                    nc.vector.tensor_tensor(out=scr0[:], in0=tau[:], in1=hi[:], op=ALU.min)
                    nc.vector.tensor_tensor(out=scr1[:], in0=tau[:], in1=lo[:], op=ALU.max)
                    nc.vector.copy_predicated(out=hi[:], mask=pred_ge[:], data=scr0[:])
                    nc.vector.copy_predicated(out=lo[:], mask=pred_lt[:], data=scr1[:])
                probe(tau, ALU.is_le, cumw)
                nc.vector.tensor_tensor(out=num[:], in0=thW[:], in1=S[:], op=ALU.subtract)
                nc.vector.reciprocal(out=rec[:], in_=cumw[:])
                nc.vector.tensor_tensor(out=scr0[:], in0=num[:], in1=rec[:], op=ALU.mult)
                nc.vector.tensor_tensor(out=scr1[:], in0=scr0[:], in1=tau[:], op=ALU.add)
                nc.vector.tensor_tensor(out=scr0[:], in0=scr1[:], in1=lo[:], op=ALU.max)
                nc.vector.tensor_tensor(out=tau[:], in0=scr0[:], in1=hi[:], op=ALU.min)
            nc.vector.tensor_copy(cand[:], tau[:])
            if infguard:
                # never-crossed columns (hi still == 2.0) -> +inf like the reference
                infs = st("infs")
                nc.vector.memset(infs[:], float("inf"))
                nc.vector.tensor_scalar(out=pred_ge[:], in0=hi[:], scalar1=2.0, scalar2=None,
                                        op0=ALU.is_ge)
                nc.vector.copy_predicated(out=cand[:], mask=pred_ge[:], data=infs[:])

            for b in range(B_LOC):
                nc.sync.dma_start(
                    out=out_loc[b].rearrange("(jb p) -> p jb", p=128),
                    in_=cand[:, b * JB:(b + 1) * JB])

    nc.compile()
    return nc


_NC_CACHE = None


def kernel(input_spikes, input_weights, input_delays, thresholds):
    global _NC_CACHE
    if _NC_CACHE is None:
        _NC_CACHE = _build()
    nc = _NC_CACHE

    s = np.ascontiguousarray(input_spikes, dtype=np.float32)
    wf = np.asarray(input_weights, dtype=np.float32)
    df = np.asarray(input_delays, dtype=np.float32)
    wT = np.ascontiguousarray(wf.T)
    dT = np.ascontiguousarray(df.T)
    th = np.ascontiguousarray(thresholds, dtype=np.float32)
    # thW[b, j] = th[j] + sum_i w[i,j]*(s[b,i] + d[i,j])
    thw = (th[None, :] + (wf * df).sum(axis=0, dtype=np.float32)[None, :]
           + s @ wf).astype(np.float32)

    in_maps = [
        dict(dT=dT, wT=wT, s_loc=np.ascontiguousarray(s[k * B_LOC:(k + 1) * B_LOC]),
             thw_in=np.ascontiguousarray(thw[k * B_LOC:(k + 1) * B_LOC]))
        for k in range(N_CORES)
    ]
    res = run_bass_kernel_spmd(nc, in_maps, core_ids=list(range(N_CORES)))
    out = np.concatenate([r["out_loc"] for r in res.results], axis=0)
    return out.astype(np.float32)


if __name__ == "__main__":
    rng = np.random.default_rng(0)
    s = rng.uniform(0, 1, (B, PRE)).astype(np.float32)
    w = (rng.normal(0, 1, (PRE, POST)) * 0.1 + 0.05).astype(np.float32)
    d = rng.uniform(0, 1, (PRE, POST)).astype(np.float32)
    th = np.ones(POST, np.float32)
    out = kernel(s, w, d, th)
    print("out", out.shape, out.dtype, np.percentile(out[np.isfinite(out)], [0, 50, 100]))


# revision 9
# speedup vs baseline: 2.1533x; 1.0320x over previous
"""Trainium2 Bass kernel for nn_EqualtimeLayer (spiking-neuron time-to-first-spike).

Math: for each (batch b, postsyn j) the output is the earliest T where
    f(T) = sum_i w[i,j] * relu(T - t[i,j]) >= theta_j,   t[i,j] = s[b,i] + d[i,j]
(first upward threshold crossing of the linear-PSP membrane potential; equivalent
to the reference's sort+cumsum+first-valid-window computation).

Device algorithm (no sort needed):
    f(tau) = sum_i w*max(t,tau) - WTtot          (one fused scalar_tensor_tensor
                                                  probe per column block, with
                                                  free-dim accumulation)
    -> bisection on the monotone predicate f(tau) >= theta, R rounds,
    -> exact finish: T* = lo + (theta + WTtot - S(lo)) / cumW(lo), clamped to
       the final bracket [lo, hi].

Sharding: data-parallel over batch, 4 batches per core on 8 cores. Weights and
delays are transposed once on the host (j-major layout) so each probe is a
per-partition-scalar op with j on partitions and i on the free axis.
"""

import numpy as np

import concourse.bacc as bacc
import concourse.mybir as mybir
import concourse.tile as tile
from concourse.bass_utils import run_bass_kernel_spmd

F32 = mybir.dt.float32
U8 = mybir.dt.uint8
ALU = mybir.AluOpType

B, PRE, POST = 32, 1024, 1024
N_CORES = 8
B_LOC = B // N_CORES          # 4 batches per core
JB = POST // 128              # 8 j-blocks of 128 partitions
NCOL = B_LOC * JB             # 32 state columns, col = b*JB + jb
R_BISECT = 5                  # coarse bracket, then Newton steps
K_NEWTON = 3


def _build(R=R_BISECT, infguard=True):
    nc = bacc.Bacc("TRN2", target_bir_lowering=False, debug=False)

    dT = nc.dram_tensor("dT", [POST, PRE], F32, kind="ExternalInput")      # d transposed [j, i]
    wT = nc.dram_tensor("wT", [POST, PRE], F32, kind="ExternalInput")      # w transposed [j, i]
    s_loc = nc.dram_tensor("s_loc", [B_LOC, PRE], F32, kind="ExternalInput")
    thw_in = nc.dram_tensor("thw_in", [B_LOC, POST], F32, kind="ExternalInput")
    out_loc = nc.dram_tensor("out_loc", [B_LOC, POST], F32, kind="ExternalOutput")

    with tile.TileContext(nc) as tc:
        with (
            tc.tile_pool(name="big", bufs=1) as big,
            tc.tile_pool(name="mat", bufs=1) as mat,
            tc.tile_pool(name="midp", bufs=2) as midp,
            tc.tile_pool(name="small", bufs=1) as small,
        ):
            # ---- load d^T (per-jb slots), build t^T[b] = d^T + s[b] ----
            # w^T reuses each jb slot as soon as that jb's t-builds finish, so the
            # w^T DMA pipelines with the t-build instead of waiting for all of it.
            dw = [mat.tile([128, PRE], F32, tag=f"dw{jb}", name=f"dT{jb}") for jb in range(JB)]
            for jb in range(JB):
                nc.sync.dma_start(out=dw[jb][:], in_=dT[jb * 128:(jb + 1) * 128, :])

            tT = []
            for b in range(B_LOC):
                tT.append(big.tile([128, JB, PRE], F32, tag=f"tT{b}", name=f"tT{b}"))

            sreps = []
            for b in range(B_LOC):
                srep = midp.tile([128, PRE], F32, tag=f"srep{b % 2}", name=f"srep{b}")
                nc.sync.dma_start(out=srep[:], in_=s_loc[b:b + 1, :].partition_broadcast(128))
                sreps.append(srep)
            for jb in range(JB):
                for b in range(B_LOC):
                    nc.vector.tensor_tensor(
                        out=tT[b][:, jb, :], in0=dw[jb][:], in1=sreps[b][:], op=ALU.add)

            wTt_tiles = [mat.tile([128, PRE], F32, tag=f"dw{jb}", name=f"wT{jb}") for jb in range(JB)]
            for jb in range(JB):
                nc.sync.dma_start(out=wTt_tiles[jb][:], in_=wT[jb * 128:(jb + 1) * 128, :])

            # ---- per-column state [128, NCOL], col = b*JB + jb ----
            def st(tag, dt=F32):
                return small.tile([128, NCOL], dt, tag=tag, name=tag)

            lo, hi, mid, S, thW = st("lo"), st("hi"), st("mid"), st("S"), st("thW")
            pred_ge, pred_lt = st("pge", U8), st("plt", U8)
            scr0 = st("scr0")

            def probe(scalar_tile, op0, acc_tile):
                """acc[:, col] = sum_i (t^T[b,jb] op0 scalar[col]) * w^T[jb]"""
                for b in range(B_LOC):
                    for jb in range(JB):
                        col = b * JB + jb
                        scratch = midp.tile([128, PRE], F32, tag="scratch", name="scratch")
                        nc.vector.scalar_tensor_tensor(
                            out=scratch[:],
                            in0=tT[b][:, jb, :],
                            scalar=scalar_tile[:, col:col + 1],
                            in1=wTt_tiles[jb][:],
                            op0=op0, op1=ALU.mult,
                            accum_out=acc_tile[:, col:col + 1])

            # thW = th + sum_i w*t, computed on the host (GEMM) and loaded directly
            # into the state layout [128, col] (col = b*JB + jb).
            for b in range(B_LOC):
                nc.sync.dma_start(
                    out=thW[:, b * JB:(b + 1) * JB],
                    in_=thw_in[b].rearrange("(jb p) -> p jb", p=128))

            # ---- bisection (coarse bracket) ----
            nc.vector.memset(lo[:], 0.0)
            nc.vector.memset(hi[:], 2.0)
            for _ in range(R):
                nc.vector.tensor_tensor(out=scr0[:], in0=lo[:], in1=hi[:], op=ALU.add)
                nc.vector.tensor_scalar_mul(mid[:], scr0[:], 0.5)
                probe(mid, ALU.max, S)
                nc.vector.tensor_tensor(out=pred_ge[:], in0=S[:], in1=thW[:], op=ALU.is_ge)
                nc.vector.tensor_tensor(out=pred_lt[:], in0=S[:], in1=thW[:], op=ALU.is_lt)
                nc.vector.copy_predicated(out=hi[:], mask=pred_ge[:], data=mid[:])
                nc.vector.copy_predicated(out=lo[:], mask=pred_lt[:], data=mid[:])

            # ---- Newton-finish: tau <- clamp(tau + (thW - S(tau))/cumW(tau), [lo,hi]) ----
            # step 1 reuses the last bisection round's S(mid); tau starts at mid.
            cumw, num, rec, cand = st("cumw"), st("num"), st("rec"), st("cand")
            tau, scr1 = st("tau"), st("scr1")
            nc.vector.tensor_copy(tau[:], mid[:])
            for k in range(K_NEWTON):
                if k > 0:
                    probe(tau, ALU.max, S)
                    nc.vector.tensor_tensor(out=pred_ge[:], in0=S[:], in1=thW[:], op=ALU.is_ge)
                    nc.vector.tensor_tensor(out=pred_lt[:], in0=S[:], in1=thW[:], op=ALU.is_lt)
                    nc.vector.tensor_tensor(out=scr0[:], in0=tau[:], in1=hi[:], op=ALU.min)
                    nc.vector.tensor_tensor(out=scr1[:], in0=tau[:], in1=lo[:], op=ALU.max)
                    nc.vector.copy_predicated(out=hi[:], mask=pred_ge[:], data=scr0[:])
                    nc.vector.copy_predicated(out=lo[:], mask=pred_lt[:], data=scr1[:])
                probe(tau, ALU.is_le, cumw)
                nc.vector.tensor_tensor(out=num[:], in0=thW[:], in1=S[:], op=ALU.subtract)
                nc.vector.reciprocal(out=rec[:], in_=cumw[:])
                nc.vector.tensor_tensor(out=scr0[:], in0=num[:], in1=rec[:], op=ALU.mult)
                nc.vector.tensor_tensor(out=scr1[:], in0=scr0[:], in1=tau[:], op=ALU.add)
                nc.vector.tensor_tensor(out=scr0[:], in0=scr1[:], in1=lo[:], op=ALU.max)
                nc.vector.tensor_tensor(out=tau[:], in0=scr0[:], in1=hi[:], op=ALU.min)
            nc.vector.tensor_copy(cand[:], tau[:])
            if infguard:
                # never-crossed columns (hi still == 2.0) -> +inf like the reference
                infs = st("infs")
                nc.vector.memset(infs[:], float("inf"))
                nc.vector.tensor_scalar(out=pred_ge[:], in0=hi[:], scalar1=2.0, scalar2=None,
                                        op0=ALU.is_ge)
                nc.vector.copy_predicated(out=cand[:], mask=pred_ge[:], data=infs[:])

            for b in range(B_LOC):
                nc.sync.dma_start(
                    out=out_loc[b].rearrange("(jb p) -> p jb", p=128),
                    in_=cand[:, b * JB:(b + 1) * JB])

    nc.compile()
    return nc


_NC_CACHE = None


def kernel(input_spikes, input_weights, input_delays, thresholds):
    global _NC_CACHE
    if _NC_CACHE is None:
        _NC_CACHE = _build()
    nc = _NC_CACHE

    s = np.ascontiguousarray(input_spikes, dtype=np.float32)
    wf = np.asarray(input_weights, dtype=np.float32)
    df = np.asarray(input_delays, dtype=np.float32)
    wT = np.ascontiguousarray(wf.T)
    dT = np.ascontiguousarray(df.T)
    th = np.ascontiguousarray(thresholds, dtype=np.float32)
    # thW[b, j] = th[j] + sum_i w[i,j]*(s[b,i] + d[i,j])
    thw = (th[None, :] + (wf * df).sum(axis=0, dtype=np.float32)[None, :]
           + s @ wf).astype(np.float32)

    in_maps = [
        dict(dT=dT, wT=wT, s_loc=np.ascontiguousarray(s[k * B_LOC:(k + 1) * B_LOC]),
             thw_in=np.ascontiguousarray(thw[k * B_LOC:(k + 1) * B_LOC]))
        for k in range(N_CORES)
    ]
    res = run_bass_kernel_spmd(nc, in_maps, core_ids=list(range(N_CORES)))
    out = np.concatenate([r["out_loc"] for r in res.results], axis=0)
    return out.astype(np.float32)


if __name__ == "__main__":
    rng = np.random.default_rng(0)
    s = rng.uniform(0, 1, (B, PRE)).astype(np.float32)
    w = (rng.normal(0, 1, (PRE, POST)) * 0.1 + 0.05).astype(np.float32)
    d = rng.uniform(0, 1, (PRE, POST)).astype(np.float32)
    th = np.ones(POST, np.float32)
    out = kernel(s, w, d, th)
    print("out", out.shape, out.dtype, np.percentile(out[np.isfinite(out)], [0, 50, 100]))
